# revision 1
# baseline (speedup 1.0000x reference)
"""DIMKT scan kernel for 8x Trainium2 NeuronCores (Bass/Tile).

Data-parallel over batch (64 rows/core). Host packs derived weight tables
(weight-side transforms only); device gathers per-token rows, transposes them
into PSUM as gate accumulation bases, and runs the sequential scan with
5 small matmuls + 2 strided sigmoids (tanh(x) = 2*sigmoid(2x) - 1) per step.
y_t = sigmoid(dot(x_{t+1}, h_t)) via a ones-column matmul batched per chunk.
"""
import numpy as np

B, S, D = 512, 500, 128
NQ, NC, NQD, NCD = 10000, 500, 100, 100
NCORES = 8
BC = B // NCORES          # 64 batch rows per core
CH = 4                    # timesteps per chunk
NSTEP = S - 1             # 499 scan steps
NCHUNK = (NSTEP + CH - 1) // CH   # 125 (last chunk has 3 steps)
XTOK = S * BC             # 32000 x tokens per core
GTOK = 128                # tokens per gather group
NGRP = XTOK // GTOK       # 250 groups

_cache = {}


def _host_pack(Eq, Ec, Eqd, Ecd, Ecorr, Wx, bx, Wsdf1, bsdf1, Wsdf2, bsdf2,
               Wpka1, bpka1, Wpka2, bpka2, Wki, bki):
    f32 = np.float32
    Wx0, Wx1, Wx2, Wx3 = (np.asarray(Wx[i * D:(i + 1) * D], f32) for i in range(4))
    T_q = np.asarray(Eq, f32) @ Wx0
    T_c = np.asarray(Ec, f32) @ Wx1 + np.asarray(bx, f32)
    A = np.asarray(Eqd, f32) @ Wx2            # [100,128]
    Bt = np.asarray(Ecd, f32) @ Wx3           # [100,128]
    T_qdcd = (A[:, None, :] + Bt[None, :, :]).reshape(NQD * NCD, D).astype(f32)
    # COMB[(qd*200 + cd*2 + co)] rows: [ki_part | pka1_part | 2*pka2_part]
    KI_qd = np.asarray(Eqd, f32) @ np.asarray(Wki[2 * D:3 * D], f32)
    KI_cd = np.asarray(Ecd, f32) @ np.asarray(Wki[3 * D:4 * D], f32)
    KI_co = np.asarray(Ecorr, f32) @ np.asarray(Wki[D:2 * D], f32) + np.asarray(bki, f32)
    P1_co = np.asarray(Ecorr, f32) @ np.asarray(Wpka1[D:2 * D], f32) + np.asarray(bpka1, f32)
    P2_co = 2.0 * (np.asarray(Ecorr, f32) @ np.asarray(Wpka2[D:2 * D], f32) + np.asarray(bpka2, f32))
    ki = (KI_qd[:, None, None, :] + KI_cd[None, :, None, :] + KI_co[None, None, :, :])
    ki = ki.reshape(NQD * NCD * 2, D)
    p1 = np.broadcast_to(P1_co[None, None, :, :], (NQD, NCD, 2, D)).reshape(-1, D)
    p2 = np.broadcast_to(P2_co[None, None, :, :], (NQD, NCD, 2, D)).reshape(-1, D)
    COMB = np.concatenate([ki, p1, p2], axis=1).astype(f32)   # [20000, 384]
    return dict(
        T_q=np.ascontiguousarray(T_q, f32),
        T_c=np.ascontiguousarray(T_c, f32),
        T_qdcd=np.ascontiguousarray(T_qdcd, f32),
        COMB=np.ascontiguousarray(COMB, f32),
        Wsdf1p=np.ascontiguousarray(Wsdf1, f32),          # +Wsdf1 (x side)
        Wsdf2p2=np.ascontiguousarray(2.0 * Wsdf2, f32),   # +2*Wsdf2 (x side)
        W1n=np.ascontiguousarray(-np.asarray(Wsdf1, f32)),
        W2n2=np.ascontiguousarray(-2.0 * np.asarray(Wsdf2, f32)),
        Wk1=np.ascontiguousarray(Wki[0:D], f32),
        Wp1=np.ascontiguousarray(Wpka1[0:D], f32),
        Wp2x2=np.ascontiguousarray(2.0 * np.asarray(Wpka2[0:D], f32)),
    )


def _group_idx(arr_sb):   # [nsteps, BC] step-major -> [128, NGRP] int32 (pad 0)
    flat = arr_sb.reshape(-1)
    pad = NGRP * GTOK - flat.shape[0]
    if pad:
        flat = np.concatenate([flat, np.zeros(pad, flat.dtype)])
    return np.ascontiguousarray(flat.reshape(NGRP, GTOK).T.astype(np.int32))


def _build_program():
    import concourse.bacc as bacc
    import concourse.bass as bass
    import concourse.mybir as mybir
    from concourse.tile import TileContext
    from concourse.masks import make_identity

    f32 = mybir.dt.float32
    Alu = mybir.AluOpType
    Act = mybir.ActivationFunctionType
    nc = bacc.Bacc("TRN2", target_bir_lowering=False, debug=False,
                   num_devices=NCORES, num_swdge_queues=4)

    dram = {}
    for nm, shape, dt in [
        ("T_q", (NQ, D), f32), ("T_c", (NC, D), f32), ("T_qdcd", (NQD * NCD, D), f32),
        ("COMB", (NQD * NCD * 2, 3 * D), f32),
        ("Wsdf1p", (D, D), f32), ("Wsdf2p2", (D, D), f32), ("W1n", (D, D), f32),
        ("W2n2", (D, D), f32), ("Wk1", (D, D), f32), ("Wp1", (D, D), f32),
        ("Wp2x2", (D, D), f32), ("h0T", (D, BC), f32),
        ("qidx", (128, NGRP), mybir.dt.int32), ("cidx", (128, NGRP), mybir.dt.int32),
        ("qdcdidx", (128, NGRP), mybir.dt.int32), ("combidx", (128, NGRP), mybir.dt.int32),
    ]:
        dram[nm] = nc.dram_tensor(nm, shape, dt, kind="ExternalInput")
    t_y = nc.dram_tensor("y", (NCHUNK * CH * BC,), f32, kind="ExternalOutput")

    def gather(out_ap, table, idx_col, queue, accum=False):
        inst = nc.gpsimd.indirect_dma_start(
            out=out_ap, out_offset=None, in_=dram[table].ap(),
            in_offset=bass.IndirectOffsetOnAxis(ap=idx_col, axis=0),
            compute_op=Alu.add if accum else Alu.bypass,
        )
        inst.ins.queue = f"qPoolDynamic{queue or ''}"
        return inst

    with TileContext(nc) as tc:
        with (
            tc.tile_pool(name="const", bufs=1) as cpool,
            tc.tile_pool(name="gath", bufs=3) as gpool,
            tc.tile_pool(name="xt", bufs=3) as xtpool,
            tc.tile_pool(name="step", bufs=3) as spool,
            tc.tile_pool(name="hpool", bufs=3) as hpool,
            tc.tile_pool(name="ppsum", bufs=2, space="PSUM") as ppool,
            tc.tile_pool(name="xpsum", bufs=2, space="PSUM") as xppool,
        ):
            ident = cpool.tile([128, 128], f32)
            make_identity(nc, ident)
            ones_col = cpool.tile([128, 1], f32)
            nc.vector.memset(ones_col[:], 1.0)
            w_sb = {}
            for nm in ["Wsdf1p", "Wsdf2p2", "W1n", "W2n2", "Wk1", "Wp1", "Wp2x2"]:
                w_sb[nm] = cpool.tile([D, D], f32, name=nm, tag=nm)
                nc.sync.dma_start(out=w_sb[nm][:], in_=dram[nm].ap())
            idx_sb = {}
            for nm in ["qidx", "cidx", "qdcdidx", "combidx"]:
                idx_sb[nm] = cpool.tile([128, NGRP], mybir.dt.int32, name=nm, tag=nm)
                nc.sync.dma_start(out=idx_sb[nm][:], in_=dram[nm].ap())
            h = hpool.tile([D, BC], f32, tag="h")
            nc.sync.dma_start(out=h[:], in_=dram["h0T"].ap())

            # deferred y state: (prod_tile, cp_base_ap, h_at_boundary, nst_prev, k_prev)
            pending = None

            for k in range(NCHUNK):
                nst = min(CH, NSTEP - k * CH)
                g0 = 2 * k
                # ---- gathers (token-major rows); one tile per group so each
                # consumer waits on exactly one DMA-queue proc ----
                xgs, cgs = [], []
                for g in range(2):
                    xg = gpool.tile([128, D], f32, tag=f"xg{g}")
                    gather(xg[:], "T_q", idx_sb["qidx"][:, g0 + g:g0 + g + 1], g % 2)
                    gather(xg[:], "T_c", idx_sb["cidx"][:, g0 + g:g0 + g + 1], g % 2, accum=True)
                    gather(xg[:], "T_qdcd", idx_sb["qdcdidx"][:, g0 + g:g0 + g + 1], g % 2, accum=True)
                    xgs.append(xg)
                    cg = gpool.tile([128, 3 * D], f32, tag=f"cg{g}")
                    gather(cg[:], "COMB", idx_sb["combidx"][:, g0 + g:g0 + g + 1], 2 + (g % 2))
                    cgs.append(cg)

                # ---- x^T via PE transpose -> psum -> sbuf ----
                xps = xppool.tile([128, 2 * D], f32, tag="xps")
                for g in range(2):
                    nc.tensor.transpose(out=xps[:, g * D:(g + 1) * D],
                                        in_=xgs[g][:], identity=ident[:])
                xT = xtpool.tile([128, 2 * D], f32, tag="xT")
                nc.vector.tensor_copy(xT[:], xps[:])

                # ---- flush previous chunk's boundary prod + y ----
                if pending is not None:
                    pprod, pct, pco, pca, ph, pnst, pk = pending
                    nc.gpsimd.tensor_tensor(out=pprod[:, (CH - 1) * 64:CH * 64],
                                            in0=ph[:], in1=xT[:, 0:64], op=Alu.mult)
                    nc.tensor.matmul(bass.AP(pct, pco + 1280, [[pca[0][0], 1], [1, 64 * pnst]]),
                                     ones_col[:], pprod[:, 0:64 * pnst],
                                     start=False, stop=True, skip_group_check=True)
                    ysb = spool.tile([1, 256], f32, tag="ysb")
                    nc.scalar.activation(ysb[:1, 0:64 * pnst],
                                         bass.AP(pct, pco + 1280, [[pca[0][0], 1], [1, 64 * pnst]]),
                                         Act.Sigmoid)
                    nc.sync.dma_start(out=t_y.ap()[pk * CH * BC: pk * CH * BC + 64 * pnst],
                                      in_=ysb[:1, 0:64 * pnst])
                    pending = None

                # ---- chunk psum: bankA = sdf1|sdf2', bankB = ki|pka1, bankC = pka2'|y ----
                cp = ppool.tile([128, 3 * 512], f32, tag="cp")
                base = cp[:]
                ct, co, ca = base.tensor, base.offset, base.ap

                def cps(col0, ncols):
                    return bass.AP(ct, co + col0, [[ca[0][0], 128], [1, ncols]])

                for g in range(2):   # ki bases -> bankB cols 0..255 (abs 512..767)
                    nc.tensor.matmul(cps(512 + g * 128, 128),
                                     cgs[g][:, 0:D], ident[:],
                                     start=(g == 0), stop=False,
                                     is_transpose=True, skip_group_check=True)
                for g in range(2):   # pka1 bases -> bankB cols 256..511
                    nc.tensor.matmul(cps(768 + g * 128, 128),
                                     cgs[g][:, D:2 * D], ident[:],
                                     start=False, stop=False,
                                     is_transpose=True, skip_group_check=True)
                for g in range(2):   # pka2' bases -> bankC cols 0..255
                    nc.tensor.matmul(cps(1024 + g * 128, 128),
                                     cgs[g][:, 2 * D:3 * D], ident[:],
                                     start=(g == 0), stop=False,
                                     is_transpose=True, skip_group_check=True)
                # x side of sdf gates -> bankA
                nc.tensor.matmul(cps(0, 256), w_sb["Wsdf1p"][:], xT[:],
                                 start=True, stop=False, skip_group_check=True)
                nc.tensor.matmul(cps(256, 256), w_sb["Wsdf2p2"][:], xT[:],
                                 start=False, stop=False, skip_group_check=True)

                prod = spool.tile([128, 256], f32, tag="prod")

                for s in range(nst):
                    nc.tensor.matmul(cps(0 + s * 64, 64), w_sb["W1n"][:], h[:],
                                     start=False, stop=False, skip_group_check=True)
                    nc.tensor.matmul(cps(256 + s * 64, 64), w_sb["W2n2"][:], h[:],
                                     start=False, stop=False, skip_group_check=True)
                    nc.tensor.matmul(cps(512 + s * 64, 64), w_sb["Wk1"][:], h[:],
                                     start=False, stop=False, skip_group_check=True)
                    gates1 = spool.tile([128, 192], f32, tag="gates1")
                    a1src = bass.AP(ct, co + s * 64, [[ca[0][0], 128], [256, 3], [1, 64]])
                    a1dst = gates1[:].rearrange("p (a b) -> p a b", b=64)
                    nc.scalar.activation(a1dst, a1src, Act.Sigmoid)
                    s1, s2p, gam = gates1[:, 0:64], gates1[:, 64:128], gates1[:, 128:192]
                    m = spool.tile([128, 64], f32, tag="m")
                    nc.vector.scalar_tensor_tensor(out=m[:], in0=s2p, scalar=2.0, in1=s1,
                                                   op0=Alu.mult, op1=Alu.mult)
                    sdf = spool.tile([128, 64], f32, tag="sdf")
                    nc.vector.tensor_tensor(out=sdf[:], in0=m[:], in1=s1, op=Alu.subtract)
                    nc.tensor.matmul(cps(768 + s * 64, 64), w_sb["Wp1"][:], sdf[:],
                                     start=False, stop=False, skip_group_check=True)
                    nc.tensor.matmul(cps(1024 + s * 64, 64), w_sb["Wp2x2"][:], sdf[:],
                                     start=False, stop=False, skip_group_check=True)
                    gates2 = spool.tile([128, 128], f32, tag="gates2")
                    a2src = bass.AP(ct, co + 768 + s * 64, [[ca[0][0], 128], [256, 2], [1, 64]])
                    a2dst = gates2[:].rearrange("p (a b) -> p a b", b=64)
                    nc.scalar.activation(a2dst, a2src, Act.Sigmoid)
                    p1, p2p = gates2[:, 0:64], gates2[:, 64:128]
                    m2 = spool.tile([128, 64], f32, tag="m2")
                    nc.vector.scalar_tensor_tensor(out=m2[:], in0=p2p, scalar=2.0, in1=p1,
                                                   op0=Alu.mult, op1=Alu.mult)
                    pka = spool.tile([128, 64], f32, tag="pka")
                    nc.vector.tensor_tensor(out=pka[:], in0=m2[:], in1=p1, op=Alu.subtract)
                    # h' = gam*h + (1-gam)*pka
                    gamc = spool.tile([128, 64], f32, tag="gamc")
                    nc.gpsimd.tensor_scalar(out=gamc[:], in0=gam, scalar1=-1.0, scalar2=1.0,
                                            op0=Alu.mult, op1=Alu.add)
                    g1 = spool.tile([128, 64], f32, tag="g1")
                    nc.vector.tensor_tensor(out=g1[:], in0=gam, in1=h[:], op=Alu.mult)
                    u = spool.tile([128, 64], f32, tag="u")
                    nc.gpsimd.tensor_tensor(out=u[:], in0=gamc[:], in1=pka[:], op=Alu.mult)
                    hn = hpool.tile([D, BC], f32, tag="h")
                    nc.vector.tensor_tensor(out=hn[:], in0=g1[:], in1=u[:], op=Alu.add)
                    h = hn
                    if s < nst - 1 or k == NCHUNK - 1:
                        nc.gpsimd.tensor_tensor(out=prod[:, s * 64:(s + 1) * 64],
                                                in0=h[:], in1=xT[:, (s + 1) * 64:(s + 2) * 64],
                                                op=Alu.mult)

                if k == NCHUNK - 1:
                    nc.tensor.matmul(bass.AP(ct, co + 1280, [[ca[0][0], 1], [1, 64 * nst]]),
                                     ones_col[:], prod[:, 0:64 * nst],
                                     start=False, stop=True, skip_group_check=True)
                    ysb = spool.tile([1, 256], f32, tag="ysb")
                    nc.scalar.activation(ysb[:1, 0:64 * nst],
                                         bass.AP(ct, co + 1280, [[ca[0][0], 1], [1, 64 * nst]]),
                                         Act.Sigmoid)
                    nc.sync.dma_start(out=t_y.ap()[k * CH * BC: k * CH * BC + 64 * nst],
                                      in_=ysb[:1, 0:64 * nst])
                else:
                    pending = (prod, ct, co, ca, h, nst, k)
    nc.compile()
    return nc


def kernel(**inputs):
    from concourse.bass_utils import run_bass_kernel_spmd

    w = _host_pack(**{k: np.asarray(inputs[k]) for k in
                      ["Eq", "Ec", "Eqd", "Ecd", "Ecorr", "Wx", "bx", "Wsdf1", "bsdf1",
                       "Wsdf2", "bsdf2", "Wpka1", "bpka1", "Wpka2", "bpka2", "Wki", "bki"]})
    q = np.asarray(inputs["question_seq"])
    c = np.asarray(inputs["concept_seq"])
    qd = np.asarray(inputs["question_diff_seq"])
    cd = np.asarray(inputs["concept_diff_seq"])
    co = np.asarray(inputs["correct_seq"])
    h0 = np.asarray(inputs["h0"], np.float32)
    qdcd = (qd * NCD + cd).astype(np.int64)
    comb = (qd * (NCD * 2) + cd * 2 + co).astype(np.int64)

    if "nc" not in _cache:
        _cache["nc"] = _build_program()
    nc = _cache["nc"]

    in_maps = []
    for core in range(NCORES):
        rows = slice(core * BC, (core + 1) * BC)
        m = dict(w)
        m["h0T"] = np.ascontiguousarray(h0[rows].T)
        m["qidx"] = _group_idx(q[rows].T)          # [S, BC] step-major
        m["cidx"] = _group_idx(c[rows].T)
        m["qdcdidx"] = _group_idx(qdcd[rows].T)
        m["combidx"] = _group_idx(comb[rows].T[:NSTEP])
        in_maps.append(m)

    global _last_in_maps
    _last_in_maps = in_maps
    res = run_bass_kernel_spmd(nc, in_maps, list(range(NCORES)))
    y = np.zeros((B, S), np.float32)
    for core in range(NCORES):
        yd = res.results[core]["y"][:NSTEP * BC].reshape(NSTEP, BC)
        y[core * BC:(core + 1) * BC, :NSTEP] = yd.T
    return y



# revision 4
# speedup vs baseline: 1.1992x; 1.1992x over previous
"""DIMKT scan kernel for 8x Trainium2 NeuronCores (Bass/Tile), v2.

Data-parallel over batch (64 rows/core), split into two 32-col streams that
run half a step out of phase so each engine's in-order queue interleaves the
two independent recurrence chains (hides cross-engine semaphore latency).
fp16 gate math (fp32 PSUM accumulation), block gathers (26 groups/instr) to
amortize the SWDGE fixed overhead, h' = h - (1-gam)*(h - pka) tail form.
"""
import numpy as np

B, S, D = 512, 500, 128
NQ, NC, NQD, NCD = 10000, 500, 100, 100
NCORES = 8
BC = B // NCORES          # 64 batch rows per core
HB = 32                   # cols per stream (2 streams)
CH = 4                    # timesteps per chunk
NSTEP = S - 1             # 499 scan steps
NCHUNK = (NSTEP + CH - 1) // CH   # 125 (last chunk has 3 steps)
XTOK = S * BC             # 32000 x tokens per core
GTOK = 128                # tokens per gather group
NGRP = XTOK // GTOK       # 250 groups
GBLK = 26                 # groups per gather block (3328 descriptors)
NBLK = (NGRP + GBLK - 1) // GBLK  # 10
NGRP_PAD = NBLK * GBLK    # 260
CHUNKS_PER_BLK = GBLK // 2  # 13

_cache = {}


def _host_pack(Eq, Ec, Eqd, Ecd, Ecorr, Wx, bx, Wsdf1, bsdf1, Wsdf2, bsdf2,
               Wpka1, bpka1, Wpka2, bpka2, Wki, bki):
    f32, f16 = np.float32, np.float16
    Wx0, Wx1, Wx2, Wx3 = (np.asarray(Wx[i * D:(i + 1) * D], f32) for i in range(4))
    T_q = np.asarray(Eq, f32) @ Wx0
    T_c = np.asarray(Ec, f32) @ Wx1 + np.asarray(bx, f32)
    A = np.asarray(Eqd, f32) @ Wx2            # [100,128]
    Bt = np.asarray(Ecd, f32) @ Wx3           # [100,128]
    T_qdcd = (A[:, None, :] + Bt[None, :, :]).reshape(NQD * NCD, D)
    # COMB[(qd*200 + cd*2 + co)] rows: [-ki_part | pka1_part | 2*pka2_part]
    # ki part negated so sigmoid gives gamc = 1-gam directly.
    KI_qd = np.asarray(Eqd, f32) @ np.asarray(Wki[2 * D:3 * D], f32)
    KI_cd = np.asarray(Ecd, f32) @ np.asarray(Wki[3 * D:4 * D], f32)
    KI_co = np.asarray(Ecorr, f32) @ np.asarray(Wki[D:2 * D], f32) + np.asarray(bki, f32)
    P1_co = np.asarray(Ecorr, f32) @ np.asarray(Wpka1[D:2 * D], f32) + np.asarray(bpka1, f32)
    P2_co = 2.0 * (np.asarray(Ecorr, f32) @ np.asarray(Wpka2[D:2 * D], f32) + np.asarray(bpka2, f32))
    ki = -(KI_qd[:, None, None, :] + KI_cd[None, :, None, :] + KI_co[None, None, :, :])
    ki = ki.reshape(NQD * NCD * 2, D)
    p1 = np.broadcast_to(P1_co[None, None, :, :], (NQD, NCD, 2, D)).reshape(-1, D)
    p2 = np.broadcast_to(P2_co[None, None, :, :], (NQD, NCD, 2, D)).reshape(-1, D)
    COMB = np.concatenate([ki, p1, p2], axis=1)   # [20000, 384]
    return dict(
        T_q=np.ascontiguousarray(T_q, f32),
        T_c=np.ascontiguousarray(T_c, f32),
        T_qdcd=np.ascontiguousarray(T_qdcd, f32),
        COMB=np.ascontiguousarray(COMB, f32),
        Wsdf1p=np.ascontiguousarray(Wsdf1, f16),          # +Wsdf1 (x side)
        Wsdf2p2=np.ascontiguousarray(2.0 * np.asarray(Wsdf2, f32), f16),
        W1n=np.ascontiguousarray(-np.asarray(Wsdf1, f32), f16),
        W2n2=np.ascontiguousarray(-2.0 * np.asarray(Wsdf2, f32), f16),
        WkN=np.ascontiguousarray(-np.asarray(Wki[0:D], f32), f16),
        Wp1=np.ascontiguousarray(Wpka1[0:D], f16),
        Wp2x2=np.ascontiguousarray(2.0 * np.asarray(Wpka2[0:D], f32), f16),
    )


def _group_idx(arr_sb):   # [nsteps, BC] step-major -> [128, NGRP_PAD] int32
    flat = arr_sb.reshape(-1)
    pad = NGRP_PAD * GTOK - flat.shape[0]
    if pad:
        flat = np.concatenate([flat, np.zeros(pad, flat.dtype)])
    return np.ascontiguousarray(flat.reshape(NGRP_PAD, GTOK).T.astype(np.int32))


def _build_program():
    import concourse.bacc as bacc
    import concourse.bass as bass
    import concourse.mybir as mybir
    from concourse.tile import TileContext
    from concourse.masks import make_identity

    f32 = mybir.dt.float32
    f16 = mybir.dt.float16
    i32 = mybir.dt.int32
    Alu = mybir.AluOpType
    Act = mybir.ActivationFunctionType
    nc = bacc.Bacc("TRN2", target_bir_lowering=False, debug=False,
                   num_devices=NCORES, num_swdge_queues=4,
                   dynamic_dma_scratch_size=65536)

    dram = {}
    for nm, shape, dt in [
        ("T_q", (NQ, D), f32), ("T_c", (NC, D), f32), ("T_qdcd", (NQD * NCD, D), f32),
        ("COMB", (NQD * NCD * 2, 3 * D), f32),
        ("Wsdf1p", (D, D), f16), ("Wsdf2p2", (D, D), f16), ("W1n", (D, D), f16),
        ("W2n2", (D, D), f16), ("WkN", (D, D), f16), ("Wp1", (D, D), f16),
        ("Wp2x2", (D, D), f16), ("h0T", (D, BC), f16),
        ("qidx", (128, NGRP_PAD), i32), ("cidx", (128, NGRP_PAD), i32),
        ("qdcdidx", (128, NGRP_PAD), i32), ("combidx", (128, NGRP_PAD), i32),
    ]:
        dram[nm] = nc.dram_tensor(nm, shape, dt, kind="ExternalInput")
    t_y = nc.dram_tensor("y", (NCHUNK * CH * BC,), f32, kind="ExternalOutput")

    def gather(out_ap, table, idx_ap, queue, accum=False):
        inst = nc.gpsimd.indirect_dma_start(
            out=out_ap, out_offset=None, in_=dram[table].ap(),
            in_offset=bass.IndirectOffsetOnAxis(ap=idx_ap, axis=0),
            compute_op=Alu.add if accum else Alu.bypass,
        )
        inst.ins.queue = f"qPoolDynamic{queue or ''}"
        return inst

    with TileContext(nc) as tc:
        with (
            tc.tile_pool(name="const", bufs=1) as cpool,
            tc.tile_pool(name="xg", bufs=2) as xgpool,
            tc.tile_pool(name="cg", bufs=2) as cgpool,
            tc.tile_pool(name="xt", bufs=3) as xtpool,
            tc.tile_pool(name="g1", bufs=4) as g1pool,
            tc.tile_pool(name="g2", bufs=4) as g2pool,
            tc.tile_pool(name="st", bufs=6) as spool,
            tc.tile_pool(name="hp", bufs=6) as hpool,
            tc.tile_pool(name="pr", bufs=2) as prodpool,
            tc.tile_pool(name="yy", bufs=2) as ypool,
            tc.tile_pool(name="ppsum", bufs=2, space="PSUM") as ppool,
            tc.tile_pool(name="xpsum", bufs=2, space="PSUM") as xppool,
        ):
            ident = cpool.tile([128, 128], f32)
            make_identity(nc, ident)
            ones_col = cpool.tile([128, 1], f16)
            nc.vector.memset(ones_col[:], 1.0)
            w_sb = {}
            for nm in ["Wsdf1p", "Wsdf2p2", "W1n", "W2n2", "WkN", "Wp1", "Wp2x2"]:
                w_sb[nm] = cpool.tile([D, D], f16, name=nm, tag=nm)
                nc.sync.dma_start(out=w_sb[nm][:], in_=dram[nm].ap())
            idx_sb = {}
            for nm in ["qidx", "cidx", "qdcdidx", "combidx"]:
                idx_sb[nm] = cpool.tile([128, NGRP_PAD], i32, name=nm, tag=nm)
                nc.sync.dma_start(out=idx_sb[nm][:], in_=dram[nm].ap())
            h0 = cpool.tile([D, BC], f16, tag="h0")
            nc.sync.dma_start(out=h0[:], in_=dram["h0T"].ap())
            h = [h0[:, 0:HB], h0[:, HB:2 * HB]]   # per-stream h APs

            xg_blk, cg_blk = {}, {}

            def issue_gathers(b, which):
                g0, g1c = b * GBLK, min((b + 1) * GBLK, NGRP_PAD)
                ng = g1c - g0
                if which == 0:
                    xg = xgpool.tile([128, GBLK * D], f32, name="xg", tag="xg")
                    xg_blk[b] = xg
                    gather(xg[:, 0:ng * D], "T_q", idx_sb["qidx"][:, g0:g1c], b % 2)
                    gather(xg[:, 0:ng * D], "T_c", idx_sb["cidx"][:, g0:g1c], b % 2, accum=True)
                    gather(xg[:, 0:ng * D], "T_qdcd", idx_sb["qdcdidx"][:, g0:g1c], b % 2, accum=True)
                else:
                    cg = cgpool.tile([128, GBLK * 3 * D], f32, name="cg", tag="cg")
                    cg_blk[b] = cg
                    gather(cg[:, 0:ng * 3 * D], "COMB", idx_sb["combidx"][:, g0:g1c], 2 + (b % 2))

            issue_gathers(0, 0)
            issue_gathers(0, 1)

            # per-stream state kept across emissions
            sdf_ap = [None, None]
            gamc_ap = [None, None]
            cp_of = {}          # chunk -> (tensor, offset, ap)
            xT_of = {}          # chunk -> xT tile
            prod_of = {}        # chunk -> prod tile

            def cps(k, col0, ncols):
                ct, co, ca = cp_of[k]
                return bass.AP(ct, co + col0, [[ca[0][0], 128], [1, ncols]])

            def emit_H1(Sm, k, s):
                c0 = s * 64 + Sm * HB
                nc.tensor.matmul(cps(k, 0 + c0, HB), w_sb["W1n"][:], h[Sm],
                                 start=False, stop=False, skip_group_check=True)
                nc.tensor.matmul(cps(k, 256 + c0, HB), w_sb["W2n2"][:], h[Sm],
                                 start=False, stop=False, skip_group_check=True)
                nc.tensor.matmul(cps(k, 512 + c0, HB), w_sb["WkN"][:], h[Sm],
                                 start=False, stop=False, skip_group_check=True)
                g1 = g1pool.tile([128, 96], f16, name=f"g1s{Sm}", tag=f"g1s{Sm}")
                ct, co, ca = cp_of[k]
                a1src = bass.AP(ct, co + c0, [[ca[0][0], 128], [256, 3], [1, HB]])
                nc.scalar.activation(g1[:].rearrange("p (a b) -> p a b", b=HB),
                                     a1src, Act.Sigmoid)
                m = spool.tile([128, HB], f16, name=f"m{Sm}", tag=f"m{Sm}")
                nc.vector.scalar_tensor_tensor(out=m[:], in0=g1[:, HB:2 * HB], scalar=2.0,
                                               in1=g1[:, 0:HB], op0=Alu.mult, op1=Alu.mult)
                sdf = spool.tile([128, HB], f16, name=f"sdf{Sm}", tag=f"sdf{Sm}")
                nc.vector.tensor_tensor(out=sdf[:], in0=m[:], in1=g1[:, 0:HB], op=Alu.subtract)
                sdf_ap[Sm] = sdf[:]
                gamc_ap[Sm] = g1[:, 2 * HB:3 * HB]

            def emit_H2(Sm, k, s, nst):
                c0 = s * 64 + Sm * HB
                nc.tensor.matmul(cps(k, 768 + c0, HB), w_sb["Wp1"][:], sdf_ap[Sm],
                                 start=False, stop=False, skip_group_check=True)
                nc.tensor.matmul(cps(k, 1024 + c0, HB), w_sb["Wp2x2"][:], sdf_ap[Sm],
                                 start=False, stop=False, skip_group_check=True)
                g2 = g2pool.tile([128, 64], f16, name=f"g2s{Sm}", tag=f"g2s{Sm}")
                ct, co, ca = cp_of[k]
                a2src = bass.AP(ct, co + 768 + c0, [[ca[0][0], 128], [256, 2], [1, HB]])
                nc.scalar.activation(g2[:].rearrange("p (a b) -> p a b", b=HB),
                                     a2src, Act.Sigmoid)
                m2 = spool.tile([128, HB], f16, name=f"m2{Sm}", tag=f"m2{Sm}")
                nc.vector.scalar_tensor_tensor(out=m2[:], in0=g2[:, HB:2 * HB], scalar=2.0,
                                               in1=g2[:, 0:HB], op0=Alu.mult, op1=Alu.mult)
                pka = spool.tile([128, HB], f16, name=f"pka{Sm}", tag=f"pka{Sm}")
                nc.vector.tensor_tensor(out=pka[:], in0=m2[:], in1=g2[:, 0:HB], op=Alu.subtract)
                d = spool.tile([128, HB], f16, name=f"d{Sm}", tag=f"d{Sm}")
                nc.vector.tensor_tensor(out=d[:], in0=h[Sm], in1=pka[:], op=Alu.subtract)
                e = spool.tile([128, HB], f16, name=f"e{Sm}", tag=f"e{Sm}")
                nc.vector.tensor_tensor(out=e[:], in0=gamc_ap[Sm], in1=d[:], op=Alu.mult)
                hn = hpool.tile([128, HB], f16, name=f"h{Sm}", tag=f"h{Sm}")
                nc.vector.tensor_tensor(out=hn[:], in0=h[Sm], in1=e[:], op=Alu.subtract)
                h[Sm] = hn[:]
                # prod: y[t] = sigma(x_{t+1} . h_t); x slot s+1
                if s < nst - 1 or k == NCHUNK - 1:
                    xn = xT_of[k][:, (s + 1) * 64 + Sm * HB:(s + 1) * 64 + Sm * HB + HB]
                elif (k + 1) in xT_of:
                    xn = xT_of[k + 1][:, Sm * HB:Sm * HB + HB]   # R stream, next chunk loaded
                else:
                    return hn  # L stream boundary: deferred to flush
                nc.gpsimd.tensor_tensor(out=prod_of[k][:, s * 64 + Sm * HB:s * 64 + Sm * HB + HB],
                                        in0=hn[:], in1=xn, op=Alu.mult)
                return hn

            hL_bound = {}  # chunk -> L h tile at boundary

            def emit_yflush(k, nst):
                # L-stream boundary prod (needs xT of chunk k+1)
                if k < NCHUNK - 1:
                    xn = xT_of[k + 1][:, 0:HB]
                    nc.vector.tensor_tensor(
                        out=prod_of[k][:, (nst - 1) * 64:(nst - 1) * 64 + HB],
                        in0=hL_bound[k][:], in1=xn, op=Alu.mult)
                ct, co, ca = cp_of[k]
                yap = bass.AP(ct, co + 1280, [[ca[0][0], 1], [1, 64 * nst]])
                nc.tensor.matmul(yap, ones_col[:], prod_of[k][:, 0:64 * nst],
                                 start=False, stop=True, skip_group_check=True)
                ysb = ypool.tile([1, 256], f32, name="ysb", tag="ysb")
                nc.scalar.activation(ysb[:1, 0:64 * nst], yap, Act.Sigmoid)
                nc.sync.dma_start(out=t_y.ap()[k * CH * BC: k * CH * BC + 64 * nst],
                                  in_=ysb[:1, 0:64 * nst])

            prev = None  # (k, s, nst) of R's pending H2

            for k in range(NCHUNK):
                nst = min(CH, NSTEP - k * CH)
                blk = (2 * k) // GBLK
                goff = ((2 * k) % GBLK) * 128

                # staggered gather issue: one block's 2 issues spread in the window
                kk = k % CHUNKS_PER_BLK
                if kk in (0, 6) and blk + 1 < NBLK:
                    issue_gathers(blk + 1, 0 if kk == 0 else 1)

                # x transpose -> psum -> sbuf (fp16)
                xps = xppool.tile([128, 256], f32, name="xps", tag="xps")
                for g in range(2):
                    nc.tensor.transpose(out=xps[:, g * 128:(g + 1) * 128],
                                        in_=xg_blk[blk][:, goff + g * 128:goff + (g + 1) * 128],
                                        identity=ident[:])
                xT = xtpool.tile([128, 256], f16, name="xT", tag="xT")
                nc.vector.tensor_copy(xT[:], xps[:])
                xT_of[k] = xT

                # chunk psum: b0 = sdf1|sdf2, b1 = ki|pka1, b2 = pka2|y
                cp = ppool.tile([128, 3 * 512], f32, name="cp", tag="cp")
                base = cp[:]
                cp_of[k] = (base.tensor, base.offset, base.ap)
                cgoff = ((2 * k) % GBLK) * 384
                cg = cg_blk[blk]
                # ki bases -> cols 512..767 (start=True resets bank1 on g==0)
                for g in range(2):
                    nc.tensor.matmul(cps(k, 512 + g * 128, 128),
                                     cg[:, cgoff + g * 384:cgoff + g * 384 + 128], ident[:],
                                     start=(g == 0), stop=False,
                                     is_transpose=True, skip_group_check=True)
                for g in range(2):   # pka1 bases -> cols 768..1023
                    nc.tensor.matmul(cps(k, 768 + g * 128, 128),
                                     cg[:, cgoff + g * 384 + 128:cgoff + g * 384 + 256], ident[:],
                                     start=False, stop=False,
                                     is_transpose=True, skip_group_check=True)
                for g in range(2):   # pka2 bases -> cols 1024..1279 (resets bank2 incl y)
                    nc.tensor.matmul(cps(k, 1024 + g * 128, 128),
                                     cg[:, cgoff + g * 384 + 256:cgoff + g * 384 + 384], ident[:],
                                     start=(g == 0), stop=False,
                                     is_transpose=True, skip_group_check=True)
                # x side of sdf gates -> bank0
                nc.tensor.matmul(cps(k, 0, 256), w_sb["Wsdf1p"][:], xT[:],
                                 start=True, stop=False, skip_group_check=True)
                nc.tensor.matmul(cps(k, 256, 256), w_sb["Wsdf2p2"][:], xT[:],
                                 start=False, stop=False, skip_group_check=True)

                prod_of[k] = prodpool.tile([128, 256], f16, name="prod", tag="prod")

                for s in range(nst):
                    emit_H1(0, k, s)
                    if prev is not None:
                        emit_H2(1, *prev)
                        prev = None
                    if s == 0 and k > 0:
                        emit_yflush(k - 1, min(CH, NSTEP - (k - 1) * CH))
                        # free old references
                        for dd in (cp_of, xT_of, prod_of, hL_bound):
                            dd.pop(k - 2, None)
                    hn = emit_H2(0, k, s, nst)
                    if s == nst - 1 and k < NCHUNK - 1:
                        hL_bound[k] = hn
                    emit_H1(1, k, s)
                    prev = (k, s, nst)

            # epilogue: R's last H2 + final flush
            emit_H2(1, *prev)
            emit_yflush(NCHUNK - 1, min(CH, NSTEP - (NCHUNK - 1) * CH))
    nc.compile()
    return nc


def kernel(**inputs):
    from concourse.bass_utils import run_bass_kernel_spmd

    w = _host_pack(**{k: np.asarray(inputs[k]) for k in
                      ["Eq", "Ec", "Eqd", "Ecd", "Ecorr", "Wx", "bx", "Wsdf1", "bsdf1",
                       "Wsdf2", "bsdf2", "Wpka1", "bpka1", "Wpka2", "bpka2", "Wki", "bki"]})
    q = np.asarray(inputs["question_seq"])
    c = np.asarray(inputs["concept_seq"])
    qd = np.asarray(inputs["question_diff_seq"])
    cd = np.asarray(inputs["concept_diff_seq"])
    co = np.asarray(inputs["correct_seq"])
    h0 = np.asarray(inputs["h0"], np.float32)
    qdcd = (qd * NCD + cd).astype(np.int64)
    comb = (qd * (NCD * 2) + cd * 2 + co).astype(np.int64)

    if "nc" not in _cache:
        _cache["nc"] = _build_program()
    nc = _cache["nc"]

    in_maps = []
    for core in range(NCORES):
        rows = slice(core * BC, (core + 1) * BC)
        m = dict(w)
        m["h0T"] = np.ascontiguousarray(h0[rows].T.astype(np.float16))
        m["qidx"] = _group_idx(q[rows].T)          # [S, BC] step-major
        m["cidx"] = _group_idx(c[rows].T)
        m["qdcdidx"] = _group_idx(qdcd[rows].T)
        m["combidx"] = _group_idx(comb[rows].T[:NSTEP])
        in_maps.append(m)

    global _last_in_maps
    _last_in_maps = in_maps
    res = run_bass_kernel_spmd(nc, in_maps, list(range(NCORES)))
    y = np.zeros((B, S), np.float32)
    for core in range(NCORES):
        yd = res.results[core]["y"][:NSTEP * BC].reshape(NSTEP, BC)
        y[core * BC:(core + 1) * BC, :NSTEP] = yd.T
    return y


# revision 5
# speedup vs baseline: 1.3672x; 1.1401x over previous
"""DIMKT scan kernel for 8x Trainium2 NeuronCores (Bass/Tile), v3.

Two 32-col batch streams per core, each with its OWN PSUM chunk tile
(2 banks) so the Tile framework's per-tile hazard tracking cannot couple the
two recurrence chains; stream-pure gather groups (group 2k+S = chunk k,
stream S) let every transpose land wholly in one stream's tile.
fp16 gate math, fp32 PSUM. Wp@(m-s1) split into Wp@m - Wp@s1 (negated weight
copies) to drop one serial DVE hop. h' = gam*h + (1-gam)*pka with gam, a on
Pool off the critical path.
"""
import numpy as np

B, S, D = 512, 500, 128
NQ, NC, NQD, NCD = 10000, 500, 100, 100
NCORES = 8
BC = B // NCORES          # 64 batch rows per core
HB = 32                   # cols per stream (2 streams)
CH = 4                    # timesteps per chunk
NSTEP = S - 1             # 499 scan steps
NCHUNK = (NSTEP + CH - 1) // CH   # 125 (last chunk has 3 steps)
XTOK = S * BC             # 32000 x tokens per core
GTOK = 128                # tokens per gather group
NGRP = XTOK // GTOK       # 250 groups
GBLK = 26                 # groups per gather block (3328 descriptors)
NBLK = (NGRP + GBLK - 1) // GBLK  # 10
NGRP_PAD = NBLK * GBLK    # 260
CHUNKS_PER_BLK = GBLK // 2  # 13

# per-stream psum regions (cols in the [128,1024] 2-bank chunk tile)
R_SDF1, R_SDF2, R_KI, R_PKA1 = 0, 128, 256, 384        # bank 0
R_PKA2, R_Y, R_XS = 512, 640, 768                      # bank 1

_cache = {}


def _host_pack(Eq, Ec, Eqd, Ecd, Ecorr, Wx, bx, Wsdf1, bsdf1, Wsdf2, bsdf2,
               Wpka1, bpka1, Wpka2, bpka2, Wki, bki):
    f32, f16 = np.float32, np.float16
    Wx0, Wx1, Wx2, Wx3 = (np.asarray(Wx[i * D:(i + 1) * D], f32) for i in range(4))
    T_q = np.asarray(Eq, f32) @ Wx0
    T_c = np.asarray(Ec, f32) @ Wx1 + np.asarray(bx, f32)
    A = np.asarray(Eqd, f32) @ Wx2
    Bt = np.asarray(Ecd, f32) @ Wx3
    T_qdcd = (A[:, None, :] + Bt[None, :, :]).reshape(NQD * NCD, D)
    # COMB rows: [-ki_part | pka1_part | 2*pka2_part]; ki negated so the
    # sigmoid yields gamc = 1-gam directly.
    KI_qd = np.asarray(Eqd, f32) @ np.asarray(Wki[2 * D:3 * D], f32)
    KI_cd = np.asarray(Ecd, f32) @ np.asarray(Wki[3 * D:4 * D], f32)
    KI_co = np.asarray(Ecorr, f32) @ np.asarray(Wki[D:2 * D], f32) + np.asarray(bki, f32)
    P1_co = np.asarray(Ecorr, f32) @ np.asarray(Wpka1[D:2 * D], f32) + np.asarray(bpka1, f32)
    P2_co = 2.0 * (np.asarray(Ecorr, f32) @ np.asarray(Wpka2[D:2 * D], f32) + np.asarray(bpka2, f32))
    ki = -(KI_qd[:, None, None, :] + KI_cd[None, :, None, :] + KI_co[None, None, :, :])
    ki = ki.reshape(NQD * NCD * 2, D)
    p1 = np.broadcast_to(P1_co[None, None, :, :], (NQD, NCD, 2, D)).reshape(-1, D)
    p2 = np.broadcast_to(P2_co[None, None, :, :], (NQD, NCD, 2, D)).reshape(-1, D)
    COMB = np.concatenate([ki, p1, p2], axis=1)   # [20000, 384]
    Wp1f = np.asarray(Wpka1[0:D], f32)
    Wp2f = 2.0 * np.asarray(Wpka2[0:D], f32)
    return dict(
        T_q=np.ascontiguousarray(T_q, f32),
        T_c=np.ascontiguousarray(T_c, f32),
        T_qdcd=np.ascontiguousarray(T_qdcd, f32),
        COMB=np.ascontiguousarray(COMB, f32),
        Wsdf1p=np.ascontiguousarray(Wsdf1, f16),
        Wsdf2p2=np.ascontiguousarray(2.0 * np.asarray(Wsdf2, f32), f16),
        W1n=np.ascontiguousarray(-np.asarray(Wsdf1, f32), f16),
        W2n2=np.ascontiguousarray(-2.0 * np.asarray(Wsdf2, f32), f16),
        WkN=np.ascontiguousarray(-np.asarray(Wki[0:D], f32), f16),
        Wp1=np.ascontiguousarray(Wp1f, f16),
        Wp1N=np.ascontiguousarray(-Wp1f, f16),
        Wp2x2=np.ascontiguousarray(Wp2f, f16),
        Wp2x2N=np.ascontiguousarray(-Wp2f, f16),
    )


def _group_idx(arr_sb):
    """[nslots, BC] step-major -> [128, NGRP_PAD] int32, stream-pure groups.

    Group g = 2k+S holds chunk k's tokens for stream S: partition
    p = t_local*32 + b maps to (slot 4k+t_local, batch col S*32+b)."""
    nslots = arr_sb.shape[0]
    pad = NCHUNK * CH - nslots
    a = arr_sb
    if pad:
        a = np.concatenate([a, np.zeros((pad, BC), a.dtype)], 0)
    a = a.reshape(NCHUNK, CH, 2, HB).transpose(0, 2, 1, 3).reshape(NGRP, GTOK)
    gpad = NGRP_PAD - NGRP
    if gpad:
        a = np.concatenate([a, np.zeros((gpad, GTOK), a.dtype)], 0)
    return np.ascontiguousarray(a.T.astype(np.int32))


def _build_program():
    import concourse.bacc as bacc
    import concourse.bass as bass
    import concourse.mybir as mybir
    from concourse.tile import TileContext
    from concourse.masks import make_identity

    f32 = mybir.dt.float32
    f16 = mybir.dt.float16
    i32 = mybir.dt.int32
    Alu = mybir.AluOpType
    Act = mybir.ActivationFunctionType
    nc = bacc.Bacc("TRN2", target_bir_lowering=False, debug=False,
                   num_devices=NCORES, num_swdge_queues=4,
                   dynamic_dma_scratch_size=65536)

    WNAMES = ["Wsdf1p", "Wsdf2p2", "W1n", "W2n2", "WkN", "Wp1", "Wp1N",
              "Wp2x2", "Wp2x2N"]
    dram = {}
    for nm, shape, dt in (
        [("T_q", (NQ, D), f32), ("T_c", (NC, D), f32),
         ("T_qdcd", (NQD * NCD, D), f32), ("COMB", (NQD * NCD * 2, 3 * D), f32)]
        + [(nm, (D, D), f16) for nm in WNAMES]
        + [("h0T", (D, BC), f16)]
        + [(nm, (128, NGRP_PAD), i32) for nm in
           ["qidx", "cidx", "qdcdidx", "combidx"]]
    ):
        dram[nm] = nc.dram_tensor(nm, shape, dt, kind="ExternalInput")
    t_y = nc.dram_tensor("y", (NCHUNK * 2 * CH * HB,), f32, kind="ExternalOutput")

    def gather(out_ap, table, idx_ap, queue, accum=False):
        inst = nc.gpsimd.indirect_dma_start(
            out=out_ap, out_offset=None, in_=dram[table].ap(),
            in_offset=bass.IndirectOffsetOnAxis(ap=idx_ap, axis=0),
            compute_op=Alu.add if accum else Alu.bypass,
        )
        inst.ins.queue = f"qPoolDynamic{queue or ''}"
        return inst

    with TileContext(nc) as tc:
        with (
            tc.tile_pool(name="const", bufs=1) as cpool,
            tc.tile_pool(name="xg", bufs=2) as xgpool,
            tc.tile_pool(name="cg", bufs=2) as cgpool,
            tc.tile_pool(name="xtL", bufs=3) as xtpoolL,
            tc.tile_pool(name="xtR", bufs=3) as xtpoolR,
            tc.tile_pool(name="g1", bufs=6) as g1pool,
            tc.tile_pool(name="g2", bufs=6) as g2pool,
            tc.tile_pool(name="st", bufs=12) as spool,
            tc.tile_pool(name="hp", bufs=6) as hpool,
            tc.tile_pool(name="prL", bufs=2) as prpoolL,
            tc.tile_pool(name="prR", bufs=2) as prpoolR,
            tc.tile_pool(name="yy", bufs=4) as ypool,
            tc.tile_pool(name="ppL", bufs=2, space="PSUM") as ppoolL,
            tc.tile_pool(name="ppR", bufs=2, space="PSUM") as ppoolR,
        ):
            ident = cpool.tile([128, 128], f32)
            make_identity(nc, ident)
            ones_col = cpool.tile([128, 1], f16)
            nc.vector.memset(ones_col[:], 1.0)
            w_sb = {}
            for nm in WNAMES:
                w_sb[nm] = cpool.tile([D, D], f16, name=nm, tag=nm)
                nc.sync.dma_start(out=w_sb[nm][:], in_=dram[nm].ap())
            idx_sb = {}
            for nm in ["qidx", "cidx", "qdcdidx", "combidx"]:
                idx_sb[nm] = cpool.tile([128, NGRP_PAD], i32, name=nm, tag=nm)
                nc.sync.dma_start(out=idx_sb[nm][:], in_=dram[nm].ap())
            h0 = cpool.tile([D, BC], f16, tag="h0")
            nc.sync.dma_start(out=h0[:], in_=dram["h0T"].ap())
            h = [h0[:, 0:HB], h0[:, HB:2 * HB]]

            xg_blk, cg_blk = {}, {}

            def issue_gathers(b, which):
                g0, g1c = b * GBLK, (b + 1) * GBLK
                if which == 0:
                    xg = xgpool.tile([128, GBLK * D], f32, name="xg", tag="xg")
                    xg_blk[b] = xg
                    gather(xg[:], "T_q", idx_sb["qidx"][:, g0:g1c], b % 2)
                    gather(xg[:], "T_c", idx_sb["cidx"][:, g0:g1c], b % 2, accum=True)
                    gather(xg[:], "T_qdcd", idx_sb["qdcdidx"][:, g0:g1c], b % 2, accum=True)
                else:
                    cg = cgpool.tile([128, GBLK * 3 * D], f32, name="cg", tag="cg")
                    cg_blk[b] = cg
                    gather(cg[:], "COMB", idx_sb["combidx"][:, g0:g1c], 2 + (b % 2))

            issue_gathers(0, 0)
            issue_gathers(0, 1)

            pools = [ppoolL, ppoolR]
            xtpools = [xtpoolL, xtpoolR]
            prpools = [prpoolL, prpoolR]
            cp_of = [{}, {}]      # [stream][chunk] -> (tensor, offset, ap)
            xT_of = [{}, {}]
            prod_of = [{}, {}]
            m_ap = [None, None]   # per-stream m tile (H1 -> H2)
            s1_ap = [None, None]
            gamc_ap = [None, None]
            a_ap = [None, None]

            def cps(Sm, k, col0, ncols):
                ct, co, ca = cp_of[Sm][k]
                return bass.AP(ct, co + col0, [[ca[0][0], 128], [1, ncols]])

            def chunk_setup(Sm, k):
                blk = (2 * k) // GBLK
                gi = (2 * k + Sm) - blk * GBLK
                cg, xg = cg_blk[blk], xg_blk[blk]
                cp = pools[Sm].tile([128, 1024], f32, name=f"cp{Sm}", tag=f"cp{Sm}")
                base = cp[:]
                cp_of[Sm][k] = (base.tensor, base.offset, base.ap)
                # bank1: pka2 bases first (start=True resets bank incl Y+XS)
                nc.tensor.matmul(cps(Sm, k, R_PKA2, 128),
                                 cg[:, gi * 384 + 256:gi * 384 + 384], ident[:],
                                 start=True, stop=False,
                                 is_transpose=True, skip_group_check=True)
                nc.tensor.matmul(cps(Sm, k, R_XS, 128),
                                 xg[:, gi * 128:(gi + 1) * 128], ident[:],
                                 start=False, stop=False,
                                 is_transpose=True, skip_group_check=True)
                xT = xtpools[Sm].tile([128, 128], f16, name=f"xT{Sm}", tag=f"xT{Sm}")
                nc.vector.tensor_copy(xT[:], bass.AP(base.tensor, base.offset + R_XS,
                                                     [[base.ap[0][0], 128], [1, 128]]))
                xT_of[Sm][k] = xT
                # bank0: ki bases first (start=True resets bank0)
                nc.tensor.matmul(cps(Sm, k, R_KI, 128),
                                 cg[:, gi * 384:gi * 384 + 128], ident[:],
                                 start=True, stop=False,
                                 is_transpose=True, skip_group_check=True)
                nc.tensor.matmul(cps(Sm, k, R_PKA1, 128),
                                 cg[:, gi * 384 + 128:gi * 384 + 256], ident[:],
                                 start=False, stop=False,
                                 is_transpose=True, skip_group_check=True)
                nc.tensor.matmul(cps(Sm, k, R_SDF1, 128), w_sb["Wsdf1p"][:], xT[:],
                                 start=False, stop=False, skip_group_check=True)
                nc.tensor.matmul(cps(Sm, k, R_SDF2, 128), w_sb["Wsdf2p2"][:], xT[:],
                                 start=False, stop=False, skip_group_check=True)
                prod_of[Sm][k] = prpools[Sm].tile([128, 128], f16,
                                                  name=f"pr{Sm}", tag=f"pr{Sm}")

            def emit_H1(Sm, k, s):
                c0 = s * HB
                for reg, wn in ((R_SDF1, "W1n"), (R_SDF2, "W2n2"), (R_KI, "WkN")):
                    nc.tensor.matmul(cps(Sm, k, reg + c0, HB), w_sb[wn][:], h[Sm],
                                     start=False, stop=False, skip_group_check=True)
                g1 = g1pool.tile([128, 96], f16, name=f"g1s{Sm}", tag=f"g1s{Sm}")
                ct, co, ca = cp_of[Sm][k]
                a1src = bass.AP(ct, co + c0, [[ca[0][0], 128], [128, 3], [1, HB]])
                nc.scalar.activation(g1[:].rearrange("p (a b) -> p a b", b=HB),
                                     a1src, Act.Sigmoid)
                m = spool.tile([128, HB], f16, name=f"m{Sm}", tag=f"m{Sm}")
                nc.vector.scalar_tensor_tensor(out=m[:], in0=g1[:, HB:2 * HB], scalar=2.0,
                                               in1=g1[:, 0:HB], op0=Alu.mult, op1=Alu.mult)
                m_ap[Sm] = m[:]
                s1_ap[Sm] = g1[:, 0:HB]
                gamc_ap[Sm] = g1[:, 2 * HB:3 * HB]
                # off-path: gam = 1-gamc, a = gam*h  (Pool)
                gm = spool.tile([128, HB], f16, name=f"gm{Sm}", tag=f"gm{Sm}")
                nc.gpsimd.tensor_scalar(out=gm[:], in0=gamc_ap[Sm], scalar1=-1.0,
                                        scalar2=1.0, op0=Alu.mult, op1=Alu.add)
                av = spool.tile([128, HB], f16, name=f"av{Sm}", tag=f"av{Sm}")
                nc.gpsimd.tensor_tensor(out=av[:], in0=gm[:], in1=h[Sm], op=Alu.mult)
                a_ap[Sm] = av[:]

            def emit_H2(Sm, k, s, nst):
                c0 = s * HB
                # Wp @ sdf = Wp@m - Wp@s1 (weights pre-negated)
                nc.tensor.matmul(cps(Sm, k, R_PKA1 + c0, HB), w_sb["Wp1"][:], m_ap[Sm],
                                 start=False, stop=False, skip_group_check=True)
                nc.tensor.matmul(cps(Sm, k, R_PKA1 + c0, HB), w_sb["Wp1N"][:], s1_ap[Sm],
                                 start=False, stop=False, skip_group_check=True)
                nc.tensor.matmul(cps(Sm, k, R_PKA2 + c0, HB), w_sb["Wp2x2"][:], m_ap[Sm],
                                 start=False, stop=False, skip_group_check=True)
                nc.tensor.matmul(cps(Sm, k, R_PKA2 + c0, HB), w_sb["Wp2x2N"][:], s1_ap[Sm],
                                 start=False, stop=False, skip_group_check=True)
                g2 = g2pool.tile([128, 64], f16, name=f"g2s{Sm}", tag=f"g2s{Sm}")
                ct, co, ca = cp_of[Sm][k]
                a2src = bass.AP(ct, co + R_PKA1 + c0, [[ca[0][0], 128], [128, 2], [1, HB]])
                nc.scalar.activation(g2[:].rearrange("p (a b) -> p a b", b=HB),
                                     a2src, Act.Sigmoid)
                m2 = spool.tile([128, HB], f16, name=f"m2{Sm}", tag=f"m2{Sm}")
                nc.vector.scalar_tensor_tensor(out=m2[:], in0=g2[:, HB:2 * HB], scalar=2.0,
                                               in1=g2[:, 0:HB], op0=Alu.mult, op1=Alu.mult)
                pka = spool.tile([128, HB], f16, name=f"pka{Sm}", tag=f"pka{Sm}")
                nc.vector.tensor_tensor(out=pka[:], in0=m2[:], in1=g2[:, 0:HB], op=Alu.subtract)
                u = spool.tile([128, HB], f16, name=f"u{Sm}", tag=f"u{Sm}")
                nc.vector.tensor_tensor(out=u[:], in0=gamc_ap[Sm], in1=pka[:], op=Alu.mult)
                hn = hpool.tile([128, HB], f16, name=f"h{Sm}", tag=f"h{Sm}")
                nc.vector.tensor_tensor(out=hn[:], in0=a_ap[Sm], in1=u[:], op=Alu.add)
                h[Sm] = hn[:]
                if s < nst - 1 or k == NCHUNK - 1:
                    xn = xT_of[Sm][k][:, (s + 1) * HB:(s + 2) * HB]
                elif (k + 1) in xT_of[Sm]:
                    xn = xT_of[Sm][k + 1][:, 0:HB]
                else:
                    return hn   # L boundary prod deferred to flush
                nc.vector.tensor_tensor(out=prod_of[Sm][k][:, s * HB:(s + 1) * HB],
                                        in0=hn[:], in1=xn, op=Alu.mult)
                return hn

            hL_bound = {}

            def emit_yflush(Sm, k, nst):
                if k < NCHUNK - 1 and Sm == 0:
                    nc.vector.tensor_tensor(
                        out=prod_of[0][k][:, (nst - 1) * HB:nst * HB],
                        in0=hL_bound[k][:], in1=xT_of[0][k + 1][:, 0:HB], op=Alu.mult)
                ct, co, ca = cp_of[Sm][k]
                yap = bass.AP(ct, co + R_Y, [[ca[0][0], 1], [1, HB * nst]])
                nc.tensor.matmul(yap, ones_col[:], prod_of[Sm][k][:, 0:HB * nst],
                                 start=False, stop=True, skip_group_check=True)
                ysb = ypool.tile([1, 128], f32, name="ysb", tag="ysb")
                nc.scalar.activation(ysb[:1, 0:HB * nst], yap, Act.Sigmoid)
                nc.sync.dma_start(
                    out=t_y.ap()[(k * 2 + Sm) * CH * HB:(k * 2 + Sm) * CH * HB + HB * nst],
                    in_=ysb[:1, 0:HB * nst])

            prev = None

            for k in range(NCHUNK):
                nst = min(CH, NSTEP - k * CH)
                kk = k % CHUNKS_PER_BLK
                blk = (2 * k) // GBLK
                if kk in (0, 6) and blk + 1 < NBLK:
                    issue_gathers(blk + 1, 0 if kk == 0 else 1)

                chunk_setup(0, k)
                chunk_setup(1, k)

                for s in range(nst):
                    emit_H1(0, k, s)
                    if prev is not None:
                        emit_H2(1, *prev)
                        prev = None
                    if s == 0 and k > 0:
                        pn = min(CH, NSTEP - (k - 1) * CH)
                        emit_yflush(1, k - 1, pn)
                        emit_yflush(0, k - 1, pn)
                        for Sm in (0, 1):
                            for dd in (cp_of[Sm], xT_of[Sm], prod_of[Sm]):
                                dd.pop(k - 2, None)
                        hL_bound.pop(k - 2, None)
                    hn = emit_H2(0, k, s, nst)
                    if s == nst - 1 and k < NCHUNK - 1:
                        hL_bound[k] = hn
                    emit_H1(1, k, s)
                    prev = (k, s, nst)

            emit_H2(1, *prev)
            emit_yflush(1, NCHUNK - 1, min(CH, NSTEP - (NCHUNK - 1) * CH))
            emit_yflush(0, NCHUNK - 1, min(CH, NSTEP - (NCHUNK - 1) * CH))
    nc.compile()
    return nc


def kernel(**inputs):
    from concourse.bass_utils import run_bass_kernel_spmd

    w = _host_pack(**{k: np.asarray(inputs[k]) for k in
                      ["Eq", "Ec", "Eqd", "Ecd", "Ecorr", "Wx", "bx", "Wsdf1", "bsdf1",
                       "Wsdf2", "bsdf2", "Wpka1", "bpka1", "Wpka2", "bpka2", "Wki", "bki"]})
    q = np.asarray(inputs["question_seq"])
    c = np.asarray(inputs["concept_seq"])
    qd = np.asarray(inputs["question_diff_seq"])
    cd = np.asarray(inputs["concept_diff_seq"])
    co = np.asarray(inputs["correct_seq"])
    h0 = np.asarray(inputs["h0"], np.float32)
    qdcd = (qd * NCD + cd).astype(np.int64)
    comb = (qd * (NCD * 2) + cd * 2 + co).astype(np.int64)

    if "nc" not in _cache:
        _cache["nc"] = _build_program()
    nc = _cache["nc"]

    in_maps = []
    for core in range(NCORES):
        rows = slice(core * BC, (core + 1) * BC)
        m = dict(w)
        m["h0T"] = np.ascontiguousarray(h0[rows].T.astype(np.float16))
        m["qidx"] = _group_idx(q[rows].T)
        m["cidx"] = _group_idx(c[rows].T)
        m["qdcdidx"] = _group_idx(qdcd[rows].T)
        m["combidx"] = _group_idx(comb[rows].T[:NSTEP])
        in_maps.append(m)

    global _last_in_maps
    _last_in_maps = in_maps
    res = run_bass_kernel_spmd(nc, in_maps, list(range(NCORES)))
    y = np.zeros((B, S), np.float32)
    for core in range(NCORES):
        yd = res.results[core]["y"].reshape(NCHUNK, 2, CH, HB)
        for Sm in range(2):
            blockrows = slice(core * BC + Sm * HB, core * BC + (Sm + 1) * HB)
            yy = yd[:, Sm].reshape(NCHUNK * CH, HB)[:NSTEP]   # [499, 32]
            y[blockrows, :NSTEP] = yy.T
    return y
